# revision 34
# baseline (speedup 1.0000x reference)
"""Trainium2 Bass kernel for nn_DFVAE (3-stage MoE routing with sorted ids).

Strategy (N=16384, LD=512, experts (8, 6, 16), 8 cores, SPMD):
  - Data-parallel over rows with *balanced* contiguous shard cuts chosen by a
    host-side pareto DP so the padded per-stage window count (max over cores)
    is minimized at window quantum Q (Q=320: C=(8,7,8), 7360 padded rows/core
    vs 8192 for the naive equal-shard 512-row scheme).
  - Everything on device is bf16 (PSUM accumulates f32): halves DMA bytes and
    evacuation traffic vs f32/f32r; rel err ~5e-3 vs the 2e-2 gate.
  - NO data-driven DMA and NO dynamic matmul weights: the host packs per core
      * wS: one 128x2048 bf16 weight block PER SLOT (its window's expert),
        streamed by static HWDGE DMAs into an SBUF ring — lhsT APs stay
        static (hardware forbids register offsets in ldweights),
      * zS: stage-1 input windows with rows pre-gathered, so stage-1 compute
        starts ~4us in instead of waiting on a monolithic 11.6us z load,
      * bR: per-slot per-m f32 biases, one tiny resident table, static APs.
    Stage-3 outputs leave per-slot as static DMAs (issued on the ACT queue
    right after that slot's last evacuation) into an output stream the host
    reassembles by program-order replay, removing the serial store tail.
    Only the per-slot row offset r is data-driven, consumed as an engine
    register AP offset (rhs reads on PE, evacuation writes on ACT/DVE).
  - Ragged expert pieces use overlap-shifted windows, short edge pieces
    emitted first so later windows of neighboring pieces rewrite their
    overhang (engines execute writes in program order; stage-3 overlap is
    resolved by the host replay instead). A host-side replay verifier checks
    the full schedule against the ids before running; any mismatch falls
    back to a numpy path.
  - Dummy matmuls on a zeroed scratch tile warm the PE p-state ramp before
    the first real slot and keep PE busy across the two stage barriers so
    the ramp never resets to half speed.
"""
import numpy as np
import ml_dtypes

import concourse.mybir as mybir
import concourse.tile as tile
from concourse import bacc, bass_utils
from concourse.bass import ds

N = 16384
LD = 512
NCORES = 8
P = 128
KO = LD // P   # 4 contraction/output subtiles
# per-stage window quanta candidates (first feasible wins; scored below)
Q_TRIPLES = ((345, 351, 354), (352, 401, 360), (414, 351, 354),
             (345, 351, 270), (320, 320, 320), (512, 512, 512))
WROW = KO * LD  # 2048 bf16 weight elements per partition per slot
STAGE_E = (8, 6, 16)
SHMIN, SHMAX = 1024, 2816
CUT_BAND = 1200

BF16 = ml_dtypes.bfloat16

LAST_RESULTS = None  # test harness reads exec_time_ns off this
_program_cache = {}

N_WARM = 30   # PE warmup dummy matmuls before first real slot
N_BOUND = 16  # PE dummy matmuls at each stage boundary
W_BUFS = 10   # weight-stream SBUF ring depth


# ---------------------------------------------------------------- host logic

def _segments(ids):
    starts = np.flatnonzero(np.diff(ids, prepend=-1))
    ends = np.append(starts[1:], len(ids))
    return list(zip(starts.tolist(), ends.tolist(), ids[starts].tolist()))


def _stage_counts(segs_all, QS, a, b):
    """Per stage: window count for shard [a, b) at that stage's quantum."""
    out = []
    for s, segs in enumerate(segs_all):
        w = 0
        for x, y, _ in segs:
            lo, hi = max(x, a), min(y, b)
            if lo < hi:
                w += -(-(hi - lo) // QS[s])
        out.append(w)
    return tuple(out)


def _balanced_cuts(segs_all, QS):
    """Choose 8 contiguous shard cuts minimizing estimated kernel time
    (pareto DP over candidate boundary positions near the equal cuts)."""
    cand = set()
    for s, segs in enumerate(segs_all):
        for x, y, _ in segs:
            for k in range(0, 9):
                for b in (x + k * QS[s], y - k * QS[s],
                          x - k * QS[s], y + k * QS[s]):
                    cand.add(b)
    cand.update(range(0, N + 1, 128))
    lev_cand = [
        sorted(p for p in cand
               if 0 < p < N and abs(p - c * (N // NCORES)) <= CUT_BAND)
        for c in range(1, NCORES)]

    tot = _stage_counts(segs_all, QS, 0, N)
    bound = sum(-(-t // NCORES) for t in tot) + 5

    def dominated(v, keys):
        return any(u != v and all(u[i] <= v[i] for i in range(3)) for u in keys)

    levels = [{0: {(0, 0, 0): None}}]
    for c in range(NCORES):
        nxt = {}
        cands = lev_cand[c] if c < NCORES - 1 else [N]
        for pos, mset in levels[c].items():
            for b1 in cands:
                L = b1 - pos
                if L < SHMIN or L > SHMAX:
                    continue
                cc = _stage_counts(segs_all, QS, pos, b1)
                for mv in mset:
                    nm = tuple(max(mv[i], cc[i]) for i in range(3))
                    if sum(nm) > bound:
                        continue
                    nxt.setdefault(b1, {}).setdefault(nm, (pos, mv))
        levels.append({
            b1: {k: v for k, v in md.items() if not dominated(k, list(md))}
            for b1, md in nxt.items()
        })
    end = levels[NCORES].get(N)
    if not end:
        return None, None, None
    best = None
    for mv in end:
        rows = sum(mv[s] * QS[s] for s in range(3))
        slots = sum(mv)
        pe = rows * 6.67 / 1000
        dma = slots * 1.459 + (mv[0] * QS[0] + mv[2] * QS[2]) * 2 * 512 / 360e3 + 0.3
        score = max(pe, dma + 2.0)
        if best is None or score < best[0]:
            best = (score, mv)
    score, mv = best
    cuts = [N]
    cur = mv
    for c in range(NCORES, 0, -1):
        pos, cur = levels[c][cuts[-1]][cur]
        cuts.append(pos)
    return cuts[::-1], mv, score


def _windows_for_shard(segs, lo, hi, Q):
    """(local_row_start, expert) windows covering [lo, hi); short pieces
    first so later windows of neighbor pieces rewrite their overhang."""
    short, norm = [], []
    for a0, b0, e in segs:
        a, b = max(a0, lo), min(b0, hi)
        if a >= b:
            continue
        length = b - a
        if length < Q:
            # edge (or defensively interior) short piece: window clamped into
            # the shard; overhang is rewritten by neighbor pieces emitted
            # later
            w = min(max(a - lo, 0), hi - lo - Q)
            short.append((w, e))
        else:
            for i in range(length // Q):
                norm.append((a - lo + i * Q, e))
            if length % Q:
                norm.append((b - lo - Q, e))
    return short + norm


def _verify_schedule(wins, ids, lo, hi, Q):
    """Replay windows in slot order; the result must equal the ids slice."""
    arr = np.full(hi - lo, -1, np.int64)
    for r, e in wins:
        if r < 0 or r + Q > hi - lo:
            return False
        arr[r:r + Q] = e
    return bool(np.all(arr == ids[lo:hi]))


def _kernel_numpy_fallback(z, Ws, bs, ids_all):
    out = np.asarray(z, np.float32)
    for s in range(3):
        nxt = np.empty_like(out)
        ids = ids_all[s]
        for e in range(Ws[s].shape[0]):
            mask = ids == e
            if mask.any():
                nxt[mask] = np.maximum(out[mask] @ Ws[s][e] + bs[s][e], 0.0)
        out = nxt
    return out


# ------------------------------------------------------------- device program

def _build_program(C, QS, SH_BUF, grouped1, n_warm=N_WARM, n_bound=N_BOUND):
    nc = bacc.Bacc("TRN2", target_bir_lowering=False, debug=False,
                   enable_asserts=False, num_devices=NCORES)
    f32 = mybir.dt.float32
    i32 = mybir.dt.int32
    bf16 = mybir.dt.bfloat16
    ACT = mybir.EngineType.Activation
    DVE = mybir.EngineType.DVE
    PE = mybir.EngineType.PE
    Ctot = sum(C)
    # weight-slot map: with grouped1, all stage-1 compute slots share one
    # static weight tile (their windows all belong to one expert piece)
    if grouped1:
        wmap = [0] * C[0] + list(range(1, 1 + C[1] + C[2]))
    else:
        wmap = list(range(Ctot))
    WTOT = wmap[-1] + 1

    desc_t = nc.dram_tensor("desc", [1, Ctot], i32, kind="ExternalInput").ap()
    wS_t = nc.dram_tensor("wS", [WTOT * P, WROW], bf16, kind="ExternalInput").ap()
    bR_t = nc.dram_tensor("bR", [P, Ctot * KO], f32, kind="ExternalInput").ap()
    zS_t = nc.dram_tensor("zS", [C[0] * P, KO * QS[0]], bf16, kind="ExternalInput").ap()
    oS_t = nc.dram_tensor("oS", [C[2] * P, KO * QS[2]], bf16, kind="ExternalOutput").ap()

    with tile.TileContext(nc) as tc:
        with (
            tc.tile_pool(name="const", bufs=1) as cpool,
            tc.tile_pool(name="w", bufs=W_BUFS) as wpool,
            tc.tile_pool(name="psum", bufs=7, space="PSUM") as ppool,
            tc.tile_pool(name="pdum", bufs=1, space="PSUM") as pdpool,
        ):
            desc_sb = cpool.tile([1, Ctot], i32, tag="desc", name="desc_sb")
            bR = cpool.tile([P, Ctot * KO], f32, tag="bR", name="bR")
            zw = [cpool.tile([P, KO * QS[0]], bf16, tag=f"zw{j}", name=f"zw{j}")
                  for j in range(C[0])]
            ost = [cpool.tile([P, KO * QS[2]], bf16, tag=f"ost{j}", name=f"ost{j}")
                   for j in range(C[2])]
            act = [cpool.tile([P, KO, SH_BUF], bf16, tag=f"act{i}", name=f"act{i}")
                   for i in range(2)]
            scratch = cpool.tile([P, P], bf16, tag="scratch", name="scratch")
            dum_ps = pdpool.tile([P, P], f32, tag="dum", name="dum_ps")

            # prologue: static HWDGE loads; first weight+input windows first
            # so slot 0 starts ASAP, then the rest of the streams.
            nc.gpsimd.memset(scratch[:], 0.0)
            nc.sync.dma_start(desc_sb[:], desc_t)
            wtiles = []

            def load_w(j):
                w = wpool.tile([P, WROW], bf16, tag="w", name="wt")
                nc.sync.dma_start(w[:], wS_t[j * P:(j + 1) * P, :])
                wtiles.append(w)

            load_w(0)
            nc.sync.dma_start(zw[0][:], zS_t[0:P, :])
            nc.sync.dma_start(bR[:], bR_t)
            for j in range(1, C[0]):
                if len(wtiles) < min(WTOT, W_BUFS):
                    load_w(len(wtiles))
                nc.sync.dma_start(zw[j][:], zS_t[j * P:(j + 1) * P, :])
            while len(wtiles) < min(WTOT, W_BUFS):
                load_w(len(wtiles))

            def dummies(n):
                for _ in range(n):
                    nc.tensor.matmul(dum_ps[:, :P], lhsT=scratch[:, 0:P],
                                     rhs=scratch[:, 0:P], start=True, stop=True)

            dummies(n_warm)

            slot = 0
            for s in range(3):
                Q = QS[s]
                for j in range(C[s]):
                    # stream further weight slots with a ring-safe lookahead
                    # (WAR lands on matmuls two slots back, always satisfied)
                    while len(wtiles) < min(WTOT, wmap[slot] + W_BUFS - 1):
                        load_w(len(wtiles))
                    w_sb = wtiles[wmap[slot]]
                    last_slot = (s == 2 and j == C[2] - 1)
                    r_engines = ([ACT, DVE] if s == 0 else
                                 [PE, ACT, DVE] if s == 1 else [PE])
                    r_val = nc.values_load(
                        desc_sb[0:1, slot:slot + 1], engines=r_engines,
                        min_val=0, max_val=SH_BUF - Q,
                        skip_runtime_bounds_check=True,
                    )
                    for m in range(KO):
                        psum = ppool.tile([P, 512], f32, tag="ps", name="psum")
                        for k in range(KO):
                            nc.tensor.matmul(
                                psum[:, :Q],
                                lhsT=w_sb[:, k * LD + m * P:k * LD + (m + 1) * P],
                                rhs=(zw[j][:, k * Q:(k + 1) * Q] if s == 0
                                     else act[s - 1][:, k, ds(r_val, Q)]),
                                start=(k == 0),
                                stop=(k == KO - 1),
                            )
                        dest = (act[s][:, m, ds(r_val, Q)] if s < 2
                                else ost[j][:, m * Q:(m + 1) * Q])
                        bias_ap = bR[:, slot * KO + m:slot * KO + m + 1]
                        # stage 3 swaps parity so the last plane (m=3) is
                        # evacuated on ACT, whose queue also issues the store
                        on_act = (m % 2 == 0) if s < 2 else (m % 2 == 1)
                        if last_slot and m == KO - 1:
                            # final plane: split the evacuation across ACT
                            # and DVE so the tail shrinks by half an evac
                            h = Q // 2
                            nc.scalar.activation(
                                ost[j][:, m * Q:m * Q + h], psum[:, :h],
                                mybir.ActivationFunctionType.Relu,
                                bias=bias_ap,
                            )
                            nc.vector.tensor_scalar(
                                ost[j][:, m * Q + h:(m + 1) * Q],
                                psum[:, h:Q], bias_ap, 0.0,
                                mybir.AluOpType.add, mybir.AluOpType.max,
                            )
                        elif on_act:
                            nc.scalar.activation(
                                dest, psum[:, :Q],
                                mybir.ActivationFunctionType.Relu,
                                bias=bias_ap,
                            )
                        else:
                            nc.vector.tensor_scalar(
                                dest, psum[:, :Q], bias_ap, 0.0,
                                mybir.AluOpType.add, mybir.AluOpType.max,
                            )
                    if s == 2:
                        if j < C[2] - 1:
                            # store from the ACT queue: in-order right after
                            # this slot's last evacuation (m=3, on ACT via
                            # the parity swap), m0/m2 on DVE already done
                            nc.scalar.dma_start(oS_t[j * P:(j + 1) * P, :],
                                                ost[j][:])
                        else:
                            # final slot: planes 0-2 leave on the idle SP
                            # queue as soon as their evacuations land; the
                            # m=3 plane (one small transfer, both evac
                            # halves done in parallel) is the only tail
                            nc.sync.dma_start(
                                oS_t[j * P:(j + 1) * P, 0:3 * Q],
                                ost[j][:, 0:3 * Q])
                            nc.sync.dma_start(
                                oS_t[j * P:(j + 1) * P, 3 * Q:4 * Q],
                                ost[j][:, 3 * Q:4 * Q])
                    slot += 1
                if s < 2:
                    dummies(n_bound)
    nc.compile()
    return nc


# ----------------------------------------------------------------- entrypoint

def kernel(z, W_dataset, b_dataset, W_assay, b_assay, W_donor, b_donor,
           dataset_ids, assay_ids, donor_ids):
    global LAST_RESULTS

    ids_all = [
        np.asarray(dataset_ids, np.int32),
        np.asarray(assay_ids, np.int32),
        np.asarray(donor_ids, np.int32),
    ]
    Ws = [
        np.ascontiguousarray(np.asarray(W_dataset, np.float32)),
        np.ascontiguousarray(np.asarray(W_assay, np.float32)),
        np.ascontiguousarray(np.asarray(W_donor, np.float32)),
    ]
    bs = [
        np.asarray(b_dataset, np.float32),
        np.asarray(b_assay, np.float32),
        np.asarray(b_donor, np.float32),
    ]
    z = np.asarray(z, np.float32)

    if any(np.any(np.diff(ids) < 0) for ids in ids_all):
        return _kernel_numpy_fallback(z, Ws, bs, ids_all)

    segs_all = [_segments(ids) for ids in ids_all]
    best = None
    # Option A: cuts exactly at stage-1 segment boundaries (one expert piece
    # per core -> all stage-1 slots share ONE weight fetch). Needs exactly
    # NCORES stage-1 segments with legal sizes.
    if (len(segs_all[0]) == NCORES and
            all(SHMIN <= y - x <= SHMAX for x, y, _ in segs_all[0])):
        cutsA = [x for x, _, _ in segs_all[0]] + [N]
        QSA = []
        for s in range(3):
            bq = None
            for Qc in range(256, 513):
                Cc = max(
                    sum(-(-(min(y, cutsA[c + 1]) - max(x, cutsA[c])) // Qc)
                        for x, y, _ in segs_all[s]
                        if max(x, cutsA[c]) < min(y, cutsA[c + 1]))
                    for c in range(NCORES))
                if bq is None or (Cc * Qc, Cc) < bq[0]:
                    bq = ((Cc * Qc, Cc), Qc)
            QSA.append(bq[1])
        QSA = tuple(QSA)
        CA = tuple(
            max(sum(-(-(min(y, cutsA[c + 1]) - max(x, cutsA[c])) // QSA[s])
                    for x, y, _ in segs_all[s]
                    if max(x, cutsA[c]) < min(y, cutsA[c + 1]))
                for c in range(NCORES))
            for s in range(3))
        rows = sum(CA[s] * QSA[s] for s in range(3))
        slots = 1 + CA[1] + CA[2]
        pe = rows * 6.67 / 1000
        dma = slots * 1.459 + (CA[0] * QSA[0] + CA[2] * QSA[2]) * 2 * 512 / 360e3 + 0.3
        best = (max(pe, dma + 2.0), QSA, cutsA, CA, True)
    for QS in Q_TRIPLES:
        r = _balanced_cuts(segs_all, QS)
        if r[0] is None:
            continue
        if best is None or r[2] < best[0]:
            best = (r[2], QS, r[0], tuple(r[1]), False)
    if best is None:
        return _kernel_numpy_fallback(z, Ws, bs, ids_all)
    _, QS, cuts, C, grouped1 = best
    sizes = [cuts[c + 1] - cuts[c] for c in range(NCORES)]
    SH_BUF = max(sizes)

    wins = [[_windows_for_shard(segs_all[s], cuts[c], cuts[c + 1], QS[s])
             for c in range(NCORES)] for s in range(3)]
    Cchk = tuple(max(len(wins[s][c]) for c in range(NCORES)) for s in range(3))
    if Cchk != C:
        return _kernel_numpy_fallback(z, Ws, bs, ids_all)
    for s in range(3):
        for c in range(NCORES):
            w = wins[s][c]
            while len(w) < C[s]:
                w.append(w[-1])
            if not _verify_schedule(w, ids_all[s], cuts[c], cuts[c + 1], QS[s]):
                return _kernel_numpy_fallback(z, Ws, bs, ids_all)
    Ctot = sum(C)

    # grouping requires every core's stage-1 windows to share one expert
    if grouped1 and any(len({e for _, e in wins[0][c]}) != 1
                        for c in range(NCORES)):
        grouped1 = False

    key = (C, QS, SH_BUF, grouped1)
    if key not in _program_cache:
        _program_cache[key] = _build_program(C, QS, SH_BUF, grouped1)
    nc = _program_cache[key]

    zbf = z.astype(BF16)
    wpack = []  # per stage: [E, P, WROW] bf16
    for s in range(3):
        E = STAGE_E[s]
        wpack.append(np.ascontiguousarray(
            Ws[s].reshape(E, KO, P, LD).transpose(0, 2, 1, 3)
            .reshape(E, P, WROW).astype(BF16)))

    if grouped1:
        wmap = [0] * C[0] + list(range(1, 1 + C[1] + C[2]))
    else:
        wmap = list(range(Ctot))
    WTOT = wmap[-1] + 1

    in_maps = []
    for c in range(NCORES):
        desc = np.zeros((1, Ctot), np.int32)
        wS = np.zeros((WTOT * P, WROW), BF16)
        bR = np.zeros((P, Ctot * KO), np.float32)
        zS = np.zeros((C[0] * P, KO * QS[0]), BF16)
        slot = 0
        for s in range(3):
            Q = QS[s]
            for j, (r, e) in enumerate(wins[s][c]):
                desc[0, slot] = r
                w = wmap[slot]
                wS[w * P:(w + 1) * P] = wpack[s][e]
                bR[:, slot * KO:(slot + 1) * KO] = bs[s][e].reshape(KO, P).T
                if s == 0:
                    g = cuts[c] + r
                    zS[j * P:(j + 1) * P] = (
                        zbf[g:g + Q].reshape(Q, KO, P)
                        .transpose(2, 1, 0).reshape(P, KO * Q))
                slot += 1
        in_maps.append({"desc": desc, "wS": wS, "bR": bR, "zS": zS})

    res = bass_utils.run_bass_kernel_spmd(nc, in_maps, core_ids=list(range(NCORES)))
    LAST_RESULTS = res

    out = np.empty((N, LD), np.float32)
    Q3 = QS[2]
    for c in range(NCORES):
        oS = np.asarray(res.results[c]["oS"]).reshape(C[2], P, KO, Q3)
        for j, (r, _) in enumerate(wins[2][c]):
            g = cuts[c] + r
            out[g:g + Q3] = (oS[j].transpose(2, 1, 0)
                             .reshape(Q3, LD).astype(np.float32))
    return out
